# revision 38
# baseline (speedup 1.0000x reference)
"""BatchedGCN Trainium2 kernel (optimized).

Per graph (batch element):
  norms_i = ||X_i||;  A = (cos_sim > 0.3) + I ; deg = rowsum(A); d = deg^-1/2
  H1 = relu(diag(d) A diag(d) (X @ W1.T) + b1)
  H2 = diag(d) A diag(d) (H1 @ W2.T) + b2
  out = H2 / max(||H2_row||, 1e-12)

Design notes (per core: 4 graphs, weights replicated; B=32 over 8 cores):
- X ships twice: fp8 DoubleRow pair-interleaved X8 ([k, p, i, n],
  d = k*256+i*128+p) for the gram matrix, bf16 X^T for X@W1.T.  All X8
  loads go first on one queue; X^T loads follow on the same queue so the
  latency-critical gram inputs get full fabric bandwidth.
- Row norms come from the gram diagonal blocks (fp8 DR matmuls); the
  threshold comparison runs un-normalized as (G * 1/n_i > t*n_j), so only
  the bound needs norms.  t*n_j is replicated across partitions entirely
  on-chip: PE-transpose [128,8] -> [8,128], then K=8 row-selector matmuls
  (eyerows) broadcast each row -- no DRAM bounce on the critical path.
- The threshold (DVE STT, deg fused via accum) writes A straight into
  fp8 DR-packed tiles [jd, i, n]; A entries {0,1,2} are exact in fp8.
- Both propagations run as fp8 DoubleRow matmuls with compensated pairs
  Ys ~= y8 + r8 (residual also fp8, values pre-scaled x16 to clear the
  fp8 subnormal floor): half the bf16 streaming cost at ~0.1% error.
- prop2 accumulates H2^T ([dout, n]) in wide 512-col DR matmuls; per-node
  ssq comes from an all-ones matmul over partitions; the replicated ssq
  row is turned into per-partition scalars by transposing [128,128]
  blocks on the PE; PE transposes bring H2 tiles back to [n, dout] and
  the 1/norm scale is applied during eviction.
- d^-1/2 chains, biases, and weight columns avoid partition-stride-1
  DMAs (4-byte-descriptor grinds); biases load as rows + PE transpose.
- Emission is wave-pipelined and engine-balanced: diag(g)/nrep(g-1)
  interleave, gram(g)/xw1(g-1) interleave, phase_e fills tensor gaps in
  the per-half phase_f pipeline (f2 trails f1 by two halves).
"""
from contextlib import ExitStack

import ml_dtypes
import numpy as np

import concourse.bass as bass
import concourse.mybir as mybir
import concourse.tile as tile
from concourse import bacc
from concourse.bass_utils import run_bass_kernel_spmd
from concourse.masks import make_identity

B, N, D_IN, D_H, D_OUT = 32, 1024, 768, 256, 128
N_CORES = 8
BPC = B // N_CORES          # graphs per core
NT = N // 128               # 8 node row tiles
KDR = D_IN // 256           # 3 DoubleRow K-chunks over D_in
NJD = N // 256              # 4 DoubleRow K-chunks over nodes
HC = D_H // 128             # 2 hidden chunks
F32 = mybir.dt.float32
BF16 = mybir.dt.bfloat16
FP8 = mybir.dt.float8e4

KNN_THRESHOLD = 0.3
COS_EPS = 1e-8
NORM_EPS = 1e-12
ALU = mybir.AluOpType
AF = mybir.ActivationFunctionType
DR = mybir.MatmulPerfMode.DoubleRow
Y_SCALE = 16.0              # pre-scale of fp8 Ys pairs (subnormal avoidance)


def build(n_batches: int = BPC):
    nc = bacc.Bacc("TRN2", debug=False, num_devices=N_CORES)
    X8 = nc.dram_tensor("X8", [n_batches, KDR, 128, 2, N], FP8,
                        kind="ExternalInput")
    R8 = nc.dram_tensor("R8", [n_batches, KDR, 128, 2, N], FP8,
                        kind="ExternalInput")
    W18 = nc.dram_tensor("W18", [KDR, 128, 2, D_H], FP8, kind="ExternalInput")
    W1R8 = nc.dram_tensor("W1R8", [KDR, 128, 2, D_H], FP8,
                          kind="ExternalInput")
    W2T = nc.dram_tensor("W2T", [D_H, D_OUT], BF16, kind="ExternalInput")
    b1 = nc.dram_tensor("b1", [D_H], F32, kind="ExternalInput")
    b2 = nc.dram_tensor("b2", [D_OUT], F32, kind="ExternalInput")
    Y = nc.dram_tensor("Y", [n_batches, N, D_OUT], F32, kind="ExternalOutput")
    with tile.TileContext(nc) as tc, ExitStack() as ctx:
        _body(ctx, tc, X8.ap(), R8.ap(), W18.ap(), W1R8.ap(), W2T.ap(),
              b1.ap(), b2.ap(), Y.ap(), n_batches)
    nc.compile()
    return nc


def _bcast_p(ap: bass.AP, parts: int = 128) -> bass.AP:
    """Broadcast a DRAM AP across `parts` partitions (partition-stride 0)."""
    return bass.AP(tensor=ap.tensor, offset=ap.offset, ap=[[0, parts]] + list(ap.ap))


def _xdr_load_ap(Xb: bass.AP) -> bass.AP:
    """DRAM [KDR, 128, 2, N] -> SBUF [128p, KDR, 2, N] load pattern."""
    return bass.AP(tensor=Xb.tensor, offset=Xb.offset,
                   ap=[[2 * N, 128], [256 * N, KDR], [N, 2], [1, N]])


class _G:
    """Per-graph state threaded between pipeline phases."""
    __slots__ = ("X8b", "R8b", "Yb", "x8", "r8", "at", "y8", "ry", "y2", "r2",
                 "h1t", "ssqv", "nct", "rc", "nrep", "degv", "dv", "dvw",
                 "dvy", "dvb", "drep", "h2tb", "rl", "obuf")


def _body(ctx, tc, X8, R8, W18, W1R8, W2T, b1, b2, Y, n_batches):
    nc = tc.nc
    nb = n_batches

    singles = ctx.enter_context(tc.tile_pool(name="singles", bufs=1))
    xpool = ctx.enter_context(tc.tile_pool(name="xpool", bufs=nb))
    apool = ctx.enter_context(tc.tile_pool(name="apool", bufs=nb))
    ypool = ctx.enter_context(tc.tile_pool(name="ypool", bufs=nb))
    h1pool = ctx.enter_context(tc.tile_pool(name="h1pool", bufs=nb * HC))
    rppool = ctx.enter_context(tc.tile_pool(name="rppool", bufs=nb))
    bvec = ctx.enter_context(tc.tile_pool(name="bvec", bufs=nb))
    sqj = ctx.enter_context(tc.tile_pool(name="sqj", bufs=2))
    tmppool = ctx.enter_context(tc.tile_pool(name="tmppool", bufs=4))
    h2pool = ctx.enter_context(tc.tile_pool(name="h2pool", bufs=2))
    opool = ctx.enter_context(tc.tile_pool(name="opool", bufs=2))
    psA = ctx.enter_context(tc.tile_pool(name="psA", bufs=4, space="PSUM"))
    psB = ctx.enter_context(tc.tile_pool(name="psB", bufs=4, space="PSUM"))
    dramp = ctx.enter_context(tc.tile_pool(name="dramp", bufs=nb, space="DRAM"))

    # ---- one-time constants ------------------------------------------------
    ident = singles.tile([128, 128], BF16)
    make_identity(nc, ident)
    identf = singles.tile([128, 128], F32)
    make_identity(nc, identf)
    ident2 = singles.tile([128, 2, 128], FP8)
    nc.gpsimd.memset(ident2, 0.0)
    make_identity(nc, ident2[:, 0, :], nomemset=True)
    make_identity(nc, ident2[:, 1, :], nomemset=True)
    ones = singles.tile([128, 128], BF16)
    nc.gpsimd.memset(ones, 1.0)
    # eyerows[k, it, :] == 1 iff k == it: K=8 row-selector for broadcasts
    eyerows = singles.tile([NT, NT, 128], BF16)
    nc.gpsimd.memset(eyerows, 0.0)
    nc.gpsimd.affine_select(out=eyerows, in_=eyerows,
                            compare_op=mybir.AluOpType.not_equal, fill=1.0,
                            base=0, pattern=[[-1, NT], [0, 128]],
                            channel_multiplier=1)
    ceps = singles.tile([128, 1], F32)
    nc.gpsimd.memset(ceps, COS_EPS * COS_EPS)
    cneps = singles.tile([128, 1], F32)
    nc.gpsimd.memset(cneps, NORM_EPS * NORM_EPS)

    b1col = singles.tile([128, HC], F32)
    b2col = singles.tile([128, 1], F32)
    w18 = singles.tile([128, KDR, 2, D_H], FP8)
    w1r8 = singles.tile([128, KDR, 2, D_H], FP8)
    w2t = [singles.tile([128, D_OUT], BF16, tag=f"w2t{k}", name=f"w2t{k}")
           for k in range(HC)]

    def load_weights():
        # emitted after the per-graph X loads so those win the DMA queues
        nc.scalar.dma_start(out=b1col, in_=bass.AP(
            tensor=b1.tensor, offset=b1.offset, ap=[[1, 128], [128, HC]]))
        nc.scalar.dma_start(out=b2col, in_=bass.AP(
            tensor=b2.tensor, offset=b2.offset, ap=[[1, 128], [1, 1]]))
        nc.sync.dma_start(out=w18, in_=bass.AP(
            tensor=W18.tensor, offset=W18.offset,
            ap=[[2 * D_H, 128], [256 * D_H, KDR], [D_H, 2], [1, D_H]]))
        nc.sync.dma_start(out=w1r8, in_=bass.AP(
            tensor=W1R8.tensor, offset=W1R8.offset,
            ap=[[2 * D_H, 128], [256 * D_H, KDR], [D_H, 2], [1, D_H]]))
        for k in range(HC):
            nc.scalar.dma_start(out=w2t[k], in_=W2T[k * 128:(k + 1) * 128, :])

    t2 = KNN_THRESHOLD * KNN_THRESHOLD

    # ---- per-phase emitters ------------------------------------------------
    def phase_load(g: _G):
        g.x8 = xpool.tile([128, KDR, 2, N], FP8, tag="x8")
        nc.sync.dma_start(out=g.x8, in_=_xdr_load_ap(g.X8b))
        g.r8 = xpool.tile([128, KDR, 2, N], FP8, tag="r8", bufs=nb)
        nc.scalar.dma_start(out=g.r8, in_=_xdr_load_ap(g.R8b))

    def phase_norm(g: _G):
        # row norms from gram diagonal blocks; produce t*n_j (bounced to a
        # partition-replicated row) and rc_i = 1/n_i (per-partition scalars)
        g.ssqv = bvec.tile([128, NT], F32, tag="ssqv")
        for it in range(NT):
            psd = psB.tile([128, 128], F32, tag="psB", name="psd")
            blk = slice(it * 128, (it + 1) * 128)
            for k in range(KDR):
                nc.tensor.matmul(psd, lhsT=g.x8[:, k, :, blk],
                                 rhs=g.x8[:, k, :, blk],
                                 start=(k == 0), stop=(k == KDR - 1),
                                 perf_mode=DR)
            dj = sqj.tile([128, 128], BF16, tag="dj")
            nc.vector.scalar_tensor_tensor(
                out=dj, in0=psd, scalar=1.0, in1=identf,
                op0=ALU.bypass, op1=ALU.mult,
                accum_out=g.ssqv[:, it:it + 1])
        g.nct = bvec.tile([128, NT], BF16, tag="nct")
        nc.scalar.activation(out=g.nct, in_=g.ssqv, func=AF.Sqrt, scale=t2)
        nclp = bvec.tile([128, NT], F32, tag="nclp")
        nc.scalar.activation(out=nclp, in_=g.ssqv, func=AF.Sqrt, bias=ceps)
        g.rc = bvec.tile([128, NT], F32, tag="rc")
        nc.vector.reciprocal(out=g.rc, in_=nclp)
        # transpose [128, NT] -> [NT, 128] so the DRAM spill is 8 contiguous
        # rows (a partition-major spill would be a 4-byte-descriptor grind)
        psn = psB.tile([128, 128], BF16, tag="psB", name="psn")
        nc.tensor.transpose(psn[:NT, :], g.nct, ident)
        nctT = sqj.tile([NT, 128], BF16, tag="nctT")
        nc.scalar.copy(out=nctT, in_=psn[:NT, :])
        # replicate row it of nctT across all partitions with K=1 matmuls:
        # no DRAM round trip on the startup critical path
        g.nrep = rppool.tile([128, N], BF16, tag="nrep")
        for nh in range(2):
            psr = psB.tile([128, 512], F32, tag="psB", name="psr")
            for itl in range(4):
                it = nh * 4 + itl
                nc.tensor.matmul(psr[:, itl * 128:(itl + 1) * 128],
                                 lhsT=eyerows[:, it, :], rhs=nctT,
                                 start=True, stop=True)
            nc.scalar.copy(out=g.nrep[:, nh * 512:(nh + 1) * 512], in_=psr)

    def phase_gram(g: _G):
        # G row tiles -> threshold -> A in fp8 DR-packed tiles, deg fused
        g.degv = bvec.tile([128, 2 * NT], F32, tag="degv")
        for it in range(NT):
            jd, i = it // 2, it % 2
            blk = slice(it * 128, (it + 1) * 128)
            for jh in range(2):
                ps = psA.tile([128, 512], F32, tag="psA")
                for k in range(KDR):
                    nc.tensor.matmul(
                        ps, lhsT=g.x8[:, k, :, blk],
                        rhs=g.x8[:, k, :, jh * 512:(jh + 1) * 512],
                        start=(k == 0), stop=(k == KDR - 1), perf_mode=DR)
                nc.vector.scalar_tensor_tensor(
                    out=g.at[jd][:, i, jh * 512:(jh + 1) * 512], in0=ps,
                    scalar=g.rc[:, it:it + 1],
                    in1=g.nrep[:, jh * 512:(jh + 1) * 512],
                    op0=ALU.mult, op1=ALU.is_gt,
                    accum_out=g.degv[:, jh * NT + it:jh * NT + it + 1])
        for jd in range(NJD):
            # self loops: add I to both diag blocks of the DR pair in one op
            sl = g.at[jd][:, 0, 2 * jd * 128:2 * jd * 128 + 128]
            dview = bass.AP(tensor=sl.tensor, offset=sl.offset,
                            ap=[list(sl.ap[0]), [N + 128, 2], [1, 128]])
            nc.gpsimd.tensor_add(out=dview, in0=dview, in1=ident2)

        dsum = bvec.tile([128, NT], F32, tag="dsum")
        nc.vector.tensor_tensor(out=dsum, in0=g.degv[:, 0:NT],
                                in1=g.degv[:, NT:2 * NT], op=ALU.add)
        sqd = bvec.tile([128, NT], F32, tag="sqd")
        nc.scalar.activation(out=sqd, in_=dsum, func=AF.Sqrt, bias=1.0)
        g.dv = bvec.tile([128, NT], F32, tag="dv")
        nc.vector.reciprocal(out=g.dv, in_=sqd)
        g.dvw = bvec.tile([128, NT], F32, tag="dvw")
        nc.vector.tensor_scalar_mul(g.dvw, g.dv, Y_SCALE / W_SCALE)
        g.dvy = bvec.tile([128, NT], F32, tag="dvy")
        nc.vector.tensor_scalar_mul(g.dvy, g.dv, Y_SCALE)
        g.dvb = bvec.tile([128, NT], BF16, tag="dvb")
        nc.vector.tensor_scalar_mul(g.dvb, g.dv, 1.0 / Y_SCALE)

    def phase_drep(g: _G):
        # bounce d/Y_SCALE to a partition-replicated row (transposed spill,
        # emitted after xw1 so the tensor engine never waits on the chain)
        psv = psB.tile([128, 128], BF16, tag="psB", name="psv")
        nc.tensor.transpose(psv[:NT, :], g.dvb, ident)
        dvT = sqj.tile([NT, 128], BF16, tag="dvT")
        nc.scalar.copy(out=dvT, in_=psv[:NT, :])
        dscr = dramp.tile([1, N], BF16, tag="dscr")
        dflat = dscr[0]
        nc.gpsimd.dma_start(
            out=bass.AP(tensor=dflat.tensor, offset=dflat.offset,
                        ap=[[128, NT], [1, 128]]),
            in_=dvT)
        g.drep = rppool.tile([128, N], BF16, tag="drep")
        nc.gpsimd.dma_start(out=g.drep, in_=_bcast_p(dflat))

    def phase_xw1(g: _G):
        # G1 = X @ W1.T via compensated fp8: X8@W18 + X8@W1r8 + R8@W18;
        # evict d_j-scaled as fp8 pair (y8, ry) for the DR propagation.
        for it in range(NT):
            jd, i = it // 2, it % 2
            blk = slice(it * 128, (it + 1) * 128)
            ps = psB.tile([128, D_H], F32, tag="psB")
            n9 = 3 * KDR
            step = 0
            for k in range(KDR):
                for lt, rt in ((g.x8, w18), (g.x8, w1r8), (g.r8, w18)):
                    mm = nc.tensor.matmul(ps, lhsT=lt[:, k, :, blk],
                                          rhs=rt[:, k], start=(step == 0),
                                          stop=(step == n9 - 1), perf_mode=DR)
                    if lt is g.x8 and rt is w1r8:
                        # same stationary as the preceding matmul of this
                        # accumulation group: skip the redundant weight load
                        mm.ins.ldweights = False
                    step += 1
            y8sl = g.y8[:, jd, i, :]
            nc.scalar.activation(out=y8sl, in_=ps, func=AF.Copy,
                                 scale=g.dvw[:, it:it + 1])
            nc.vector.scalar_tensor_tensor(
                out=g.ry[:, jd, i, :], in0=ps, scalar=g.dvw[:, it:it + 1],
                in1=y8sl, op0=ALU.mult, op1=ALU.subtract)

    def phase_prop1(g: _G):
        # M1^T = (A diag(d) G1)^T via DR pairs; H1^T = relu(d_i * M1^T + b1)
        pss = {}
        for hc in range(HC):
            g.h1t.append(h1pool.tile([128, N], BF16, tag="h1", name="h1"))
            for ih in range(2):
                pss[hc, ih] = psA.tile([128, 512], F32, tag="psA", name="psd2")
        nsrc = 2 * NJD
        step = 0
        for jd in range(NJD):
            for src in (g.y8, g.ry):
                st = step == 0
                sp = step == nsrc - 1
                step += 1
                for hc in range(HC):
                    lhsT = src[:, jd, :, hc * 128:(hc + 1) * 128]
                    for ih in range(2):
                        nc.tensor.matmul(
                            pss[hc, ih], lhsT=lhsT,
                            rhs=g.at[jd][:, :, ih * 512:(ih + 1) * 512],
                            start=st, stop=sp, perf_mode=DR)
        for hc in range(HC):
            for ih in range(2):
                tmp = tmppool.tile([128, 512], F32, tag="tmp")
                nc.vector.tensor_tensor(out=tmp, in0=pss[hc, ih],
                                        in1=g.drep[:, ih * 512:(ih + 1) * 512],
                                        op=ALU.mult)
                nc.scalar.activation(out=g.h1t[hc][:, ih * 512:(ih + 1) * 512],
                                     in_=tmp, func=AF.Relu,
                                     bias=b1col[:, hc:hc + 1])

    def phase_e(g: _G):
        # Ys2 = d_j * (H1 @ W2.T), evicted as fp8 pair (y2, r2)
        for it in range(NT):
            jd, i = it // 2, it % 2
            ps = psB.tile([128, D_OUT], F32, tag="psB", name="psE")
            for hc in range(HC):
                nc.tensor.matmul(ps, lhsT=g.h1t[hc][:, it * 128:(it + 1) * 128],
                                 rhs=w2t[hc], start=(hc == 0),
                                 stop=(hc == HC - 1))
            y2sl = g.y2[:, jd, i, :]
            nc.scalar.activation(out=y2sl, in_=ps, func=AF.Copy,
                                 scale=g.dvy[:, it:it + 1])
            nc.vector.scalar_tensor_tensor(
                out=g.r2[:, jd, i, :], in0=ps, scalar=g.dvy[:, it:it + 1],
                in1=y2sl, op0=ALU.mult, op1=ALU.subtract)

    def phase_f1(g: _G, ih: int):
        # H2^T half = (A Ys2)^T in wide DR matmuls; per-node 1/norm derived
        # on-chip via transposes of the partition-replicated ssq row.
        if ih == 0:
            g.h2tb = h2pool.tile([128, N], BF16, tag="h2tb", bufs=nb)
            g.rl = bvec.tile([128, NT], F32, tag="rl")
        nsrc = 2 * NJD
        ps2 = psA.tile([128, 512], F32, tag="psA", name="ps2")
        step = 0
        for jd in range(NJD):
            for src in (g.y2, g.r2):
                nc.tensor.matmul(
                    ps2, lhsT=src[:, jd],
                    rhs=g.at[jd][:, :, ih * 512:(ih + 1) * 512],
                    start=(step == 0), stop=(step == nsrc - 1),
                    perf_mode=DR)
                step += 1
        half = slice(ih * 512, (ih + 1) * 512)
        tmp = tmppool.tile([128, 512], F32, tag="tmp")
        nc.vector.tensor_tensor(out=tmp, in0=ps2, in1=g.drep[:, half],
                                op=ALU.mult)
        nc.vector.tensor_scalar(out=g.h2tb[:, half], in0=tmp, scalar1=b2col,
                                scalar2=None, op0=ALU.add)
        sq = sqj.tile([128, 512], BF16, tag="sq")
        nc.vector.tensor_tensor(out=sq, in0=g.h2tb[:, half],
                                in1=g.h2tb[:, half], op=ALU.mult)
        pssq = psB.tile([128, 512], F32, tag="psB", name="pssq")
        nc.tensor.matmul(pssq, lhsT=ones, rhs=sq, start=True, stop=True)
        sqs = sqj.tile([128, 512], BF16, tag="sqs")
        nc.scalar.copy(out=sqs, in_=pssq)
        # every row of sqs is the same ssq vector, so transposing a [128,128]
        # block turns column n into the per-partition scalar layout
        psq = psB.tile([128, 512], BF16, tag="psB", name="psq")
        for itl in range(4):
            nc.tensor.transpose(psq[:, itl * 128:(itl + 1) * 128],
                                sqs[:, itl * 128:(itl + 1) * 128], ident)
        rsl = psq[:, 0:1]
        nc.scalar.copy(
            out=g.rl[:, ih * 4:(ih + 1) * 4],
            in_=bass.AP(tensor=rsl.tensor, offset=rsl.offset,
                        ap=[list(rsl.ap[0]), [128, 4]]))

    def phase_f2(g: _G, ih: int):
        # per-node 1/max(norm, eps), then PE transposes back to [n, dout]
        # with the scale applied during the DVE eviction
        srl = bvec.tile([128, NT // 2], F32, tag="srl")
        nc.scalar.activation(out=srl, in_=g.rl[:, ih * 4:(ih + 1) * 4],
                             func=AF.Sqrt, bias=cneps)
        rli = bvec.tile([128, NT // 2], F32, tag="rli")
        nc.vector.reciprocal(out=rli, in_=srl)
        if ih == 0:
            g.obuf = opool.tile([128, NT * D_OUT], F32, tag="obuf")
        for itl in range(4):
            it = ih * 4 + itl
            pst = psB.tile([128, 128], BF16, tag="psB", name="pst")
            nc.tensor.transpose(pst, g.h2tb[:, it * 128:(it + 1) * 128], ident)
            nc.scalar.activation(out=g.obuf[:, it * 128:(it + 1) * 128],
                                 in_=pst, func=AF.Copy,
                                 scale=rli[:, itl:itl + 1])
        nc.sync.dma_start(
            out=bass.AP(tensor=g.Yb.tensor,
                        offset=g.Yb.offset + ih * 512 * D_OUT,
                        ap=[[D_OUT, 128], [128 * D_OUT, NT // 2], [1, D_OUT]]),
            in_=g.obuf[:, ih * 512:(ih + 1) * 512])

    # ---- wave-pipelined driver ---------------------------------------------
    gs = []
    for bi in range(nb):
        g = _G()
        g.X8b, g.R8b, g.Yb = X8[bi], R8[bi], Y[bi]
        g.h1t = []
        g.at = []
        gs.append(g)

    for g in gs:
        phase_load(g)
    load_weights()
    for g in gs:
        # A tiles allocated up front so the threshold can write DR slices
        for jd in range(NJD):
            g.at.append(apool.tile([128, 2, N], FP8, tag="at", bufs=nb * NJD,
                                   name="at"))
        g.y8 = ypool.tile([128, NJD, 2, D_H], FP8, tag="y8")
        g.ry = ypool.tile([128, NJD, 2, D_H], FP8, tag="ry", bufs=nb)
        g.y2 = ypool.tile([128, NJD, 2, D_OUT], FP8, tag="y2", bufs=nb)
        g.r2 = ypool.tile([128, NJD, 2, D_OUT], FP8, tag="r2", bufs=nb)
    for g in gs:
        phase_norm(g)
    # xw1(g-1) is emitted between gram(g-1) and gram(g) so its DVE residual
    # evictions drain while the tensor engine streams the next graph's gram
    phase_gram(gs[0])
    for gi in range(1, nb):
        phase_xw1(gs[gi - 1])
        phase_drep(gs[gi - 1])
        phase_gram(gs[gi])
    phase_xw1(gs[nb - 1])
    phase_drep(gs[nb - 1])
    for g in gs:
        phase_prop1(g)
    # phase_e emissions are interleaved into the f pipeline as tensor-engine
    # filler while each half's normalize chain drains on vector/scalar/gpsimd
    phase_e(gs[0])
    phase_e(gs[1])
    halves = [(g, ih) for g in gs for ih in range(2)]
    nh = len(halves)
    for i in range(nh):
        phase_f1(*halves[i])
        if i % 2 == 1 and i // 2 + 2 < nb:
            phase_e(gs[i // 2 + 2])
        if i >= 2:
            phase_f2(*halves[i - 2])
    phase_f2(*halves[nh - 2])
    phase_f2(*halves[nh - 1])


# revision 39
# speedup vs baseline: 1.0239x; 1.0239x over previous
"""BatchedGCN Trainium2 kernel (optimized).

Per graph (batch element):
  norms_i = ||X_i||;  A = (cos_sim > 0.3) + I ; deg = rowsum(A); d = deg^-1/2
  H1 = relu(diag(d) A diag(d) (X @ W1.T) + b1)
  H2 = diag(d) A diag(d) (H1 @ W2.T) + b2
  out = H2 / max(||H2_row||, 1e-12)

Design notes (per core: 4 graphs, weights replicated; B=32 over 8 cores):
- X ships twice: fp8 DoubleRow pair-interleaved X8 ([k, p, i, n],
  d = k*256+i*128+p) for the gram matrix, bf16 X^T for X@W1.T.  All X8
  loads go first on one queue; X^T loads follow on the same queue so the
  latency-critical gram inputs get full fabric bandwidth.
- Row norms come from the gram diagonal blocks (fp8 DR matmuls); the
  threshold comparison runs un-normalized as (G * 1/n_i > t*n_j), so only
  the bound needs norms.  t*n_j is replicated across partitions entirely
  on-chip: PE-transpose [128,8] -> [8,128], then K=8 row-selector matmuls
  (eyerows) broadcast each row -- no DRAM bounce on the critical path.
- The threshold (DVE STT, deg fused via accum) writes A straight into
  fp8 DR-packed tiles [jd, i, n]; A entries {0,1,2} are exact in fp8.
- Both propagations run as fp8 DoubleRow matmuls with compensated pairs
  Ys ~= y8 + r8 (residual also fp8, values pre-scaled x16 to clear the
  fp8 subnormal floor): half the bf16 streaming cost at ~0.1% error.
- prop2 accumulates H2^T ([dout, n]) in wide 512-col DR matmuls; per-node
  ssq comes from an all-ones matmul over partitions; the replicated ssq
  row is turned into per-partition scalars by transposing [128,128]
  blocks on the PE; PE transposes bring H2 tiles back to [n, dout] and
  the 1/norm scale is applied during eviction.
- d^-1/2 chains, biases, and weight columns avoid partition-stride-1
  DMAs (4-byte-descriptor grinds); biases load as rows + PE transpose.
- Emission is wave-pipelined and engine-balanced: diag(g)/nrep(g-1)
  interleave, gram(g)/xw1(g-1) interleave, phase_e fills tensor gaps in
  the per-half phase_f pipeline (f2 trails f1 by two halves).
"""
from contextlib import ExitStack

import ml_dtypes
import numpy as np

import concourse.bass as bass
import concourse.mybir as mybir
import concourse.tile as tile
from concourse import bacc
from concourse.bass_utils import run_bass_kernel_spmd
from concourse.masks import make_identity

B, N, D_IN, D_H, D_OUT = 32, 1024, 768, 256, 128
N_CORES = 8
BPC = B // N_CORES          # graphs per core
NT = N // 128               # 8 node row tiles
KDR = D_IN // 256           # 3 DoubleRow K-chunks over D_in
NJD = N // 256              # 4 DoubleRow K-chunks over nodes
HC = D_H // 128             # 2 hidden chunks
F32 = mybir.dt.float32
BF16 = mybir.dt.bfloat16
FP8 = mybir.dt.float8e4

KNN_THRESHOLD = 0.3
COS_EPS = 1e-8
NORM_EPS = 1e-12
ALU = mybir.AluOpType
AF = mybir.ActivationFunctionType
DR = mybir.MatmulPerfMode.DoubleRow
Y_SCALE = 16.0              # pre-scale of fp8 Ys pairs (subnormal avoidance)


def build(n_batches: int = BPC):
    nc = bacc.Bacc("TRN2", debug=False, num_devices=N_CORES)
    X8 = nc.dram_tensor("X8", [n_batches, KDR, 128, 2, N], FP8,
                        kind="ExternalInput")
    R8 = nc.dram_tensor("R8", [n_batches, KDR, 128, 2, N], FP8,
                        kind="ExternalInput")
    W18 = nc.dram_tensor("W18", [KDR, 128, 2, D_H], FP8, kind="ExternalInput")
    W1R8 = nc.dram_tensor("W1R8", [KDR, 128, 2, D_H], FP8,
                          kind="ExternalInput")
    W2T = nc.dram_tensor("W2T", [D_H, D_OUT], BF16, kind="ExternalInput")
    b1 = nc.dram_tensor("b1", [D_H], F32, kind="ExternalInput")
    b2 = nc.dram_tensor("b2", [D_OUT], F32, kind="ExternalInput")
    Y = nc.dram_tensor("Y", [n_batches, N, D_OUT], F32, kind="ExternalOutput")
    with tile.TileContext(nc) as tc, ExitStack() as ctx:
        _body(ctx, tc, X8.ap(), R8.ap(), W18.ap(), W1R8.ap(), W2T.ap(),
              b1.ap(), b2.ap(), Y.ap(), n_batches)
    nc.compile()
    return nc


def _bcast_p(ap: bass.AP, parts: int = 128) -> bass.AP:
    """Broadcast a DRAM AP across `parts` partitions (partition-stride 0)."""
    return bass.AP(tensor=ap.tensor, offset=ap.offset, ap=[[0, parts]] + list(ap.ap))


def _xdr_load_ap(Xb: bass.AP) -> bass.AP:
    """DRAM [KDR, 128, 2, N] -> SBUF [128p, KDR, 2, N] load pattern."""
    return bass.AP(tensor=Xb.tensor, offset=Xb.offset,
                   ap=[[2 * N, 128], [256 * N, KDR], [N, 2], [1, N]])


class _G:
    """Per-graph state threaded between pipeline phases."""
    __slots__ = ("X8b", "R8b", "Yb", "x8", "r8", "at", "y8", "ry", "y2", "r2",
                 "h1t", "ssqv", "nct", "rc", "nrep", "degv", "dv", "dvw",
                 "dvy", "dvb", "drep", "h2tb", "rl", "obuf")


def _body(ctx, tc, X8, R8, W18, W1R8, W2T, b1, b2, Y, n_batches):
    nc = tc.nc
    nb = n_batches

    singles = ctx.enter_context(tc.tile_pool(name="singles", bufs=1))
    xpool = ctx.enter_context(tc.tile_pool(name="xpool", bufs=nb))
    apool = ctx.enter_context(tc.tile_pool(name="apool", bufs=nb))
    ypool = ctx.enter_context(tc.tile_pool(name="ypool", bufs=nb))
    h1pool = ctx.enter_context(tc.tile_pool(name="h1pool", bufs=nb * HC))
    rppool = ctx.enter_context(tc.tile_pool(name="rppool", bufs=nb))
    bvec = ctx.enter_context(tc.tile_pool(name="bvec", bufs=nb))
    sqj = ctx.enter_context(tc.tile_pool(name="sqj", bufs=2))
    tmppool = ctx.enter_context(tc.tile_pool(name="tmppool", bufs=4))
    h2pool = ctx.enter_context(tc.tile_pool(name="h2pool", bufs=2))
    opool = ctx.enter_context(tc.tile_pool(name="opool", bufs=2))
    psA = ctx.enter_context(tc.tile_pool(name="psA", bufs=4, space="PSUM"))
    psB = ctx.enter_context(tc.tile_pool(name="psB", bufs=4, space="PSUM"))
    dramp = ctx.enter_context(tc.tile_pool(name="dramp", bufs=nb, space="DRAM"))

    # ---- one-time constants ------------------------------------------------
    ident = singles.tile([128, 128], BF16)
    make_identity(nc, ident)
    identf = singles.tile([128, 128], F32)
    make_identity(nc, identf)
    ident2 = singles.tile([128, 2, 128], FP8)
    nc.gpsimd.memset(ident2, 0.0)
    make_identity(nc, ident2[:, 0, :], nomemset=True)
    make_identity(nc, ident2[:, 1, :], nomemset=True)
    ones = singles.tile([128, 128], BF16)
    nc.gpsimd.memset(ones, 1.0)
    # eyerows[k, it, :] == 1 iff k == it: K=8 row-selector for broadcasts
    eyerows = singles.tile([NT, NT, 128], BF16)
    nc.gpsimd.memset(eyerows, 0.0)
    nc.gpsimd.affine_select(out=eyerows, in_=eyerows,
                            compare_op=mybir.AluOpType.not_equal, fill=1.0,
                            base=0, pattern=[[-1, NT], [0, 128]],
                            channel_multiplier=1)
    ceps = singles.tile([128, 1], F32)
    nc.gpsimd.memset(ceps, COS_EPS * COS_EPS)
    cneps = singles.tile([128, 1], F32)
    nc.gpsimd.memset(cneps, NORM_EPS * NORM_EPS)

    b1col = singles.tile([128, HC], F32)
    b2col = singles.tile([128, 1], F32)
    w18 = singles.tile([128, KDR, 2, D_H], FP8)
    w1r8 = singles.tile([128, KDR, 2, D_H], FP8)
    w2t = [singles.tile([128, D_OUT], BF16, tag=f"w2t{k}", name=f"w2t{k}")
           for k in range(HC)]

    def load_weights():
        # emitted after the per-graph X loads so those win the DMA queues
        nc.scalar.dma_start(out=b1col, in_=bass.AP(
            tensor=b1.tensor, offset=b1.offset, ap=[[1, 128], [128, HC]]))
        nc.scalar.dma_start(out=b2col, in_=bass.AP(
            tensor=b2.tensor, offset=b2.offset, ap=[[1, 128], [1, 1]]))
        nc.sync.dma_start(out=w18, in_=bass.AP(
            tensor=W18.tensor, offset=W18.offset,
            ap=[[2 * D_H, 128], [256 * D_H, KDR], [D_H, 2], [1, D_H]]))
        nc.sync.dma_start(out=w1r8, in_=bass.AP(
            tensor=W1R8.tensor, offset=W1R8.offset,
            ap=[[2 * D_H, 128], [256 * D_H, KDR], [D_H, 2], [1, D_H]]))
        for k in range(HC):
            nc.scalar.dma_start(out=w2t[k], in_=W2T[k * 128:(k + 1) * 128, :])

    t2 = KNN_THRESHOLD * KNN_THRESHOLD

    # ---- per-phase emitters ------------------------------------------------
    def phase_load(g: _G):
        g.x8 = xpool.tile([128, KDR, 2, N], FP8, tag="x8")
        nc.sync.dma_start(out=g.x8, in_=_xdr_load_ap(g.X8b))
        g.r8 = xpool.tile([128, KDR, 2, N], FP8, tag="r8", bufs=nb)
        nc.scalar.dma_start(out=g.r8, in_=_xdr_load_ap(g.R8b))

    def phase_norm(g: _G):
        # row norms from gram diagonal blocks; produce t*n_j (bounced to a
        # partition-replicated row) and rc_i = 1/n_i (per-partition scalars)
        g.ssqv = bvec.tile([128, NT], F32, tag="ssqv")
        for it in range(NT):
            psd = psB.tile([128, 128], F32, tag="psB", name="psd")
            blk = slice(it * 128, (it + 1) * 128)
            for k in range(KDR):
                nc.tensor.matmul(psd, lhsT=g.x8[:, k, :, blk],
                                 rhs=g.x8[:, k, :, blk],
                                 start=(k == 0), stop=(k == KDR - 1),
                                 perf_mode=DR)
            dj = sqj.tile([128, 128], BF16, tag="dj")
            nc.vector.scalar_tensor_tensor(
                out=dj, in0=psd, scalar=1.0, in1=identf,
                op0=ALU.bypass, op1=ALU.mult,
                accum_out=g.ssqv[:, it:it + 1])
        g.nct = bvec.tile([128, NT], BF16, tag="nct")
        nc.scalar.activation(out=g.nct, in_=g.ssqv, func=AF.Sqrt, scale=t2)
        nclp = bvec.tile([128, NT], F32, tag="nclp")
        nc.scalar.activation(out=nclp, in_=g.ssqv, func=AF.Sqrt, bias=ceps)
        g.rc = bvec.tile([128, NT], F32, tag="rc")
        nc.vector.reciprocal(out=g.rc, in_=nclp)
        # transpose [128, NT] -> [NT, 128] so the DRAM spill is 8 contiguous
        # rows (a partition-major spill would be a 4-byte-descriptor grind)
        psn = psB.tile([128, 128], BF16, tag="psB", name="psn")
        nc.tensor.transpose(psn[:NT, :], g.nct, ident)
        nctT = sqj.tile([NT, 128], BF16, tag="nctT")
        nc.scalar.copy(out=nctT, in_=psn[:NT, :])
        # replicate row it of nctT across all partitions with K=1 matmuls:
        # no DRAM round trip on the startup critical path
        g.nrep = rppool.tile([128, N], BF16, tag="nrep")
        for nh in range(2):
            psr = psB.tile([128, 512], F32, tag="psB", name="psr")
            for itl in range(4):
                it = nh * 4 + itl
                nc.tensor.matmul(psr[:, itl * 128:(itl + 1) * 128],
                                 lhsT=eyerows[:, it, :], rhs=nctT,
                                 start=True, stop=True)
            nc.scalar.copy(out=g.nrep[:, nh * 512:(nh + 1) * 512], in_=psr)

    def phase_gram(g: _G):
        # G row tiles -> threshold -> A in fp8 DR-packed tiles, deg fused
        g.degv = bvec.tile([128, 2 * NT], F32, tag="degv")
        for it in range(NT):
            jd, i = it // 2, it % 2
            blk = slice(it * 128, (it + 1) * 128)
            for jh in range(2):
                ps = psA.tile([128, 512], F32, tag="psA")
                for k in range(KDR):
                    nc.tensor.matmul(
                        ps, lhsT=g.x8[:, k, :, blk],
                        rhs=g.x8[:, k, :, jh * 512:(jh + 1) * 512],
                        start=(k == 0), stop=(k == KDR - 1), perf_mode=DR)
                nc.vector.scalar_tensor_tensor(
                    out=g.at[jd][:, i, jh * 512:(jh + 1) * 512], in0=ps,
                    scalar=g.rc[:, it:it + 1],
                    in1=g.nrep[:, jh * 512:(jh + 1) * 512],
                    op0=ALU.mult, op1=ALU.is_gt,
                    accum_out=g.degv[:, jh * NT + it:jh * NT + it + 1])
        for jd in range(NJD):
            # self loops: add I to both diag blocks of the DR pair in one op
            sl = g.at[jd][:, 0, 2 * jd * 128:2 * jd * 128 + 128]
            dview = bass.AP(tensor=sl.tensor, offset=sl.offset,
                            ap=[list(sl.ap[0]), [N + 128, 2], [1, 128]])
            nc.gpsimd.tensor_add(out=dview, in0=dview, in1=ident2)

        dsum = bvec.tile([128, NT], F32, tag="dsum")
        nc.vector.tensor_tensor(out=dsum, in0=g.degv[:, 0:NT],
                                in1=g.degv[:, NT:2 * NT], op=ALU.add)
        sqd = bvec.tile([128, NT], F32, tag="sqd")
        nc.scalar.activation(out=sqd, in_=dsum, func=AF.Sqrt, bias=1.0)
        g.dv = bvec.tile([128, NT], F32, tag="dv")
        nc.vector.reciprocal(out=g.dv, in_=sqd)
        g.dvw = bvec.tile([128, NT], F32, tag="dvw")
        nc.vector.tensor_scalar_mul(g.dvw, g.dv, Y_SCALE / W_SCALE)
        g.dvy = bvec.tile([128, NT], F32, tag="dvy")
        nc.vector.tensor_scalar_mul(g.dvy, g.dv, Y_SCALE)
        g.dvb = bvec.tile([128, NT], BF16, tag="dvb")
        nc.vector.tensor_scalar_mul(g.dvb, g.dv, 1.0 / Y_SCALE)

    def phase_drep(g: _G):
        # bounce d/Y_SCALE to a partition-replicated row (transposed spill,
        # emitted after xw1 so the tensor engine never waits on the chain)
        psv = psB.tile([128, 128], BF16, tag="psB", name="psv")
        nc.tensor.transpose(psv[:NT, :], g.dvb, ident)
        dvT = sqj.tile([NT, 128], BF16, tag="dvT")
        nc.scalar.copy(out=dvT, in_=psv[:NT, :])
        dscr = dramp.tile([1, N], BF16, tag="dscr")
        dflat = dscr[0]
        nc.gpsimd.dma_start(
            out=bass.AP(tensor=dflat.tensor, offset=dflat.offset,
                        ap=[[128, NT], [1, 128]]),
            in_=dvT)
        g.drep = rppool.tile([128, N], BF16, tag="drep")
        nc.gpsimd.dma_start(out=g.drep, in_=_bcast_p(dflat))

    def phase_xw1(g: _G):
        # G1 = X @ W1.T via compensated fp8: X8@W18 + X8@W1r8 + R8@W18;
        # evict d_j-scaled as fp8 pair (y8, ry) for the DR propagation.
        for it in range(NT):
            jd, i = it // 2, it % 2
            blk = slice(it * 128, (it + 1) * 128)
            ps = psB.tile([128, D_H], F32, tag="psB")
            n9 = 3 * KDR
            step = 0
            for k in range(KDR):
                for lt, rt in ((g.x8, w18), (g.x8, w1r8), (g.r8, w18)):
                    mm = nc.tensor.matmul(ps, lhsT=lt[:, k, :, blk],
                                          rhs=rt[:, k], start=(step == 0),
                                          stop=(step == n9 - 1), perf_mode=DR)
                    if lt is g.x8 and rt is w1r8:
                        # same stationary as the preceding matmul of this
                        # accumulation group: skip the redundant weight load
                        mm.ins.ldweights = False
                    step += 1
            y8sl = g.y8[:, jd, i, :]
            nc.scalar.activation(out=y8sl, in_=ps, func=AF.Copy,
                                 scale=g.dvw[:, it:it + 1])
            nc.vector.scalar_tensor_tensor(
                out=g.ry[:, jd, i, :], in0=ps, scalar=g.dvw[:, it:it + 1],
                in1=y8sl, op0=ALU.mult, op1=ALU.subtract)

    def phase_prop1(g: _G):
        # M1^T = (A diag(d) G1)^T via DR pairs; H1^T = relu(d_i * M1^T + b1)
        pss = {}
        for hc in range(HC):
            g.h1t.append(h1pool.tile([128, N], BF16, tag="h1", name="h1"))
            for ih in range(2):
                pss[hc, ih] = psA.tile([128, 512], F32, tag="psA", name="psd2")
        nsrc = 2 * NJD
        step = 0
        for jd in range(NJD):
            for src in (g.y8, g.ry):
                st = step == 0
                sp = step == nsrc - 1
                step += 1
                for hc in range(HC):
                    lhsT = src[:, jd, :, hc * 128:(hc + 1) * 128]
                    for ih in range(2):
                        nc.tensor.matmul(
                            pss[hc, ih], lhsT=lhsT,
                            rhs=g.at[jd][:, :, ih * 512:(ih + 1) * 512],
                            start=st, stop=sp, perf_mode=DR)
        for hc in range(HC):
            for ih in range(2):
                tmp = tmppool.tile([128, 512], F32, tag="tmp")
                nc.vector.tensor_tensor(out=tmp, in0=pss[hc, ih],
                                        in1=g.drep[:, ih * 512:(ih + 1) * 512],
                                        op=ALU.mult)
                nc.scalar.activation(out=g.h1t[hc][:, ih * 512:(ih + 1) * 512],
                                     in_=tmp, func=AF.Relu,
                                     bias=b1col[:, hc:hc + 1])

    def phase_e(g: _G):
        # Ys2 = d_j * (H1 @ W2.T), evicted as fp8 pair (y2, r2)
        for it in range(NT):
            jd, i = it // 2, it % 2
            ps = psB.tile([128, D_OUT], F32, tag="psB", name="psE")
            for hc in range(HC):
                nc.tensor.matmul(ps, lhsT=g.h1t[hc][:, it * 128:(it + 1) * 128],
                                 rhs=w2t[hc], start=(hc == 0),
                                 stop=(hc == HC - 1))
            y2sl = g.y2[:, jd, i, :]
            nc.scalar.activation(out=y2sl, in_=ps, func=AF.Copy,
                                 scale=g.dvy[:, it:it + 1])
            nc.vector.scalar_tensor_tensor(
                out=g.r2[:, jd, i, :], in0=ps, scalar=g.dvy[:, it:it + 1],
                in1=y2sl, op0=ALU.mult, op1=ALU.subtract)

    def phase_f1(g: _G, ih: int):
        # H2^T half = (A Ys2)^T in wide DR matmuls; per-node 1/norm derived
        # on-chip via transposes of the partition-replicated ssq row.
        if ih == 0:
            g.h2tb = h2pool.tile([128, N], BF16, tag="h2tb", bufs=nb)
            g.rl = bvec.tile([128, NT], F32, tag="rl")
        nsrc = 2 * NJD
        ps2 = psA.tile([128, 512], F32, tag="psA", name="ps2")
        step = 0
        for jd in range(NJD):
            for src in (g.y2, g.r2):
                nc.tensor.matmul(
                    ps2, lhsT=src[:, jd],
                    rhs=g.at[jd][:, :, ih * 512:(ih + 1) * 512],
                    start=(step == 0), stop=(step == nsrc - 1),
                    perf_mode=DR)
                step += 1
        half = slice(ih * 512, (ih + 1) * 512)
        tmp = tmppool.tile([128, 512], F32, tag="tmp")
        nc.vector.tensor_tensor(out=tmp, in0=ps2, in1=g.drep[:, half],
                                op=ALU.mult)
        nc.vector.tensor_scalar(out=g.h2tb[:, half], in0=tmp, scalar1=b2col,
                                scalar2=None, op0=ALU.add)
        sq = sqj.tile([128, 512], BF16, tag="sq")
        nc.vector.tensor_tensor(out=sq, in0=g.h2tb[:, half],
                                in1=g.h2tb[:, half], op=ALU.mult)
        pssq = psB.tile([128, 512], F32, tag="psB", name="pssq")
        nc.tensor.matmul(pssq, lhsT=ones, rhs=sq, start=True, stop=True)
        sqs = sqj.tile([128, 512], BF16, tag="sqs")
        nc.scalar.copy(out=sqs, in_=pssq)
        # every row of sqs is the same ssq vector, so transposing a [128,128]
        # block turns column n into the per-partition scalar layout
        psq = psB.tile([128, 512], BF16, tag="psB", name="psq")
        for itl in range(4):
            nc.tensor.transpose(psq[:, itl * 128:(itl + 1) * 128],
                                sqs[:, itl * 128:(itl + 1) * 128], ident)
        rsl = psq[:, 0:1]
        nc.scalar.copy(
            out=g.rl[:, ih * 4:(ih + 1) * 4],
            in_=bass.AP(tensor=rsl.tensor, offset=rsl.offset,
                        ap=[list(rsl.ap[0]), [128, 4]]))

    def phase_f2(g: _G, ih: int):
        # per-node 1/max(norm, eps), then PE transposes back to [n, dout]
        # with the scale applied during the DVE eviction
        srl = bvec.tile([128, NT // 2], F32, tag="srl")
        nc.scalar.activation(out=srl, in_=g.rl[:, ih * 4:(ih + 1) * 4],
                             func=AF.Sqrt, bias=cneps)
        rli = bvec.tile([128, NT // 2], F32, tag="rli")
        nc.vector.reciprocal(out=rli, in_=srl)
        if ih == 0:
            g.obuf = opool.tile([128, NT * D_OUT], F32, tag="obuf")
        for itl in range(4):
            it = ih * 4 + itl
            pst = psB.tile([128, 128], BF16, tag="psB", name="pst")
            nc.tensor.transpose(pst, g.h2tb[:, it * 128:(it + 1) * 128], ident)
            nc.vector.tensor_scalar(out=g.obuf[:, it * 128:(it + 1) * 128],
                                    in0=pst, scalar1=rli[:, itl:itl + 1],
                                    scalar2=None, op0=ALU.mult)
        nc.sync.dma_start(
            out=bass.AP(tensor=g.Yb.tensor,
                        offset=g.Yb.offset + ih * 512 * D_OUT,
                        ap=[[D_OUT, 128], [128 * D_OUT, NT // 2], [1, D_OUT]]),
            in_=g.obuf[:, ih * 512:(ih + 1) * 512])

    # ---- wave-pipelined driver ---------------------------------------------
    gs = []
    for bi in range(nb):
        g = _G()
        g.X8b, g.R8b, g.Yb = X8[bi], R8[bi], Y[bi]
        g.h1t = []
        g.at = []
        gs.append(g)

    for g in gs:
        phase_load(g)
    load_weights()
    for g in gs:
        # A tiles allocated up front so the threshold can write DR slices
        for jd in range(NJD):
            g.at.append(apool.tile([128, 2, N], FP8, tag="at", bufs=nb * NJD,
                                   name="at"))
        g.y8 = ypool.tile([128, NJD, 2, D_H], FP8, tag="y8")
        g.ry = ypool.tile([128, NJD, 2, D_H], FP8, tag="ry", bufs=nb)
        g.y2 = ypool.tile([128, NJD, 2, D_OUT], FP8, tag="y2", bufs=nb)
        g.r2 = ypool.tile([128, NJD, 2, D_OUT], FP8, tag="r2", bufs=nb)
    for g in gs:
        phase_norm(g)
    # xw1(g-1) is emitted between gram(g-1) and gram(g) so its DVE residual
    # evictions drain while the tensor engine streams the next graph's gram
    phase_gram(gs[0])
    for gi in range(1, nb):
        phase_xw1(gs[gi - 1])
        phase_drep(gs[gi - 1])
        phase_gram(gs[gi])
    phase_xw1(gs[nb - 1])
    phase_drep(gs[nb - 1])
    for g in gs:
        phase_prop1(g)
    # phase_e emissions are interleaved into the f pipeline as tensor-engine
    # filler while each half's normalize chain drains on vector/scalar/gpsimd
    phase_e(gs[0])
    phase_e(gs[1])
    halves = [(g, ih) for g in gs for ih in range(2)]
    nh = len(halves)
    for i in range(nh):
        phase_f1(*halves[i])
        if i % 2 == 1 and i // 2 + 2 < nb:
            phase_e(gs[i // 2 + 2])
        if i >= 2:
            phase_f2(*halves[i - 2])
    phase_f2(*halves[nh - 2])
    phase_f2(*halves[nh - 1])


# revision 41
# speedup vs baseline: 1.0365x; 1.0123x over previous
"""BatchedGCN Trainium2 kernel (optimized).

Per graph (batch element):
  norms_i = ||X_i||;  A = (cos_sim > 0.3) + I ; deg = rowsum(A); d = deg^-1/2
  H1 = relu(diag(d) A diag(d) (X @ W1.T) + b1)
  H2 = diag(d) A diag(d) (H1 @ W2.T) + b2
  out = H2 / max(||H2_row||, 1e-12)

Design notes (per core: 4 graphs, weights replicated; B=32 over 8 cores):
- X ships twice: fp8 DoubleRow pair-interleaved X8 ([k, p, i, n],
  d = k*256+i*128+p) for the gram matrix, bf16 X^T for X@W1.T.  All X8
  loads go first on one queue; X^T loads follow on the same queue so the
  latency-critical gram inputs get full fabric bandwidth.
- Row norms come from the gram diagonal blocks (fp8 DR matmuls); the
  threshold comparison runs un-normalized as (G * 1/n_i > t*n_j), so only
  the bound needs norms.  t*n_j is replicated across partitions entirely
  on-chip: PE-transpose [128,8] -> [8,128], then K=8 row-selector matmuls
  (eyerows) broadcast each row -- no DRAM bounce on the critical path.
- The threshold (DVE STT, deg fused via accum) writes A straight into
  fp8 DR-packed tiles [jd, i, n]; A entries {0,1,2} are exact in fp8.
- Both propagations run as fp8 DoubleRow matmuls with compensated pairs
  Ys ~= y8 + r8 (residual also fp8, values pre-scaled x16 to clear the
  fp8 subnormal floor): half the bf16 streaming cost at ~0.1% error.
- prop2 accumulates H2^T ([dout, n]) in wide 512-col DR matmuls; per-node
  ssq comes from an all-ones matmul over partitions; the replicated ssq
  row is turned into per-partition scalars by transposing [128,128]
  blocks on the PE; PE transposes bring H2 tiles back to [n, dout] and
  the 1/norm scale is applied during eviction.
- d^-1/2 chains, biases, and weight columns avoid partition-stride-1
  DMAs (4-byte-descriptor grinds); biases load as rows + PE transpose.
- Emission is wave-pipelined and engine-balanced: diag(g)/nrep(g-1)
  interleave, gram(g)/xw1(g-1) interleave, phase_e fills tensor gaps in
  the per-half phase_f pipeline (f2 trails f1 by two halves).
"""
from contextlib import ExitStack

import ml_dtypes
import numpy as np

import concourse.bass as bass
import concourse.mybir as mybir
import concourse.tile as tile
from concourse import bacc
from concourse.bass_utils import run_bass_kernel_spmd
from concourse.masks import make_identity

B, N, D_IN, D_H, D_OUT = 32, 1024, 768, 256, 128
N_CORES = 8
BPC = B // N_CORES          # graphs per core
NT = N // 128               # 8 node row tiles
KDR = D_IN // 256           # 3 DoubleRow K-chunks over D_in
NJD = N // 256              # 4 DoubleRow K-chunks over nodes
HC = D_H // 128             # 2 hidden chunks
F32 = mybir.dt.float32
BF16 = mybir.dt.bfloat16
FP8 = mybir.dt.float8e4

KNN_THRESHOLD = 0.3
COS_EPS = 1e-8
NORM_EPS = 1e-12
ALU = mybir.AluOpType
AF = mybir.ActivationFunctionType
DR = mybir.MatmulPerfMode.DoubleRow
Y_SCALE = 16.0              # pre-scale of fp8 Ys pairs (subnormal avoidance)


def build(n_batches: int = BPC):
    nc = bacc.Bacc("TRN2", debug=False, num_devices=N_CORES)
    X8 = nc.dram_tensor("X8", [n_batches, KDR, 128, 2, N], FP8,
                        kind="ExternalInput")
    R8 = nc.dram_tensor("R8", [n_batches, KDR, 128, 2, N], FP8,
                        kind="ExternalInput")
    W18 = nc.dram_tensor("W18", [KDR, 128, 2, D_H], FP8, kind="ExternalInput")
    W1R8 = nc.dram_tensor("W1R8", [KDR, 128, 2, D_H], FP8,
                          kind="ExternalInput")
    W2T = nc.dram_tensor("W2T", [D_H, D_OUT], BF16, kind="ExternalInput")
    b1 = nc.dram_tensor("b1", [D_H], F32, kind="ExternalInput")
    b2 = nc.dram_tensor("b2", [D_OUT], F32, kind="ExternalInput")
    Y = nc.dram_tensor("Y", [n_batches, N, D_OUT], F32, kind="ExternalOutput")
    with tile.TileContext(nc) as tc, ExitStack() as ctx:
        _body(ctx, tc, X8.ap(), R8.ap(), W18.ap(), W1R8.ap(), W2T.ap(),
              b1.ap(), b2.ap(), Y.ap(), n_batches)
    nc.compile()
    return nc


def _bcast_p(ap: bass.AP, parts: int = 128) -> bass.AP:
    """Broadcast a DRAM AP across `parts` partitions (partition-stride 0)."""
    return bass.AP(tensor=ap.tensor, offset=ap.offset, ap=[[0, parts]] + list(ap.ap))


def _xdr_load_ap(Xb: bass.AP) -> bass.AP:
    """DRAM [KDR, 128, 2, N] -> SBUF [128p, KDR, 2, N] load pattern."""
    return bass.AP(tensor=Xb.tensor, offset=Xb.offset,
                   ap=[[2 * N, 128], [256 * N, KDR], [N, 2], [1, N]])


class _G:
    """Per-graph state threaded between pipeline phases."""
    __slots__ = ("X8b", "R8b", "Yb", "x8", "r8", "at", "y8", "ry", "y2", "r2",
                 "h1t", "ssqv", "nct", "rc", "nrep", "degv", "dv", "dvw",
                 "dvy", "dvb", "drep", "h2tb", "rl", "obuf")


def _body(ctx, tc, X8, R8, W18, W1R8, W2T, b1, b2, Y, n_batches):
    nc = tc.nc
    nb = n_batches

    singles = ctx.enter_context(tc.tile_pool(name="singles", bufs=1))
    xpool = ctx.enter_context(tc.tile_pool(name="xpool", bufs=nb))
    apool = ctx.enter_context(tc.tile_pool(name="apool", bufs=nb))
    ypool = ctx.enter_context(tc.tile_pool(name="ypool", bufs=nb))
    h1pool = ctx.enter_context(tc.tile_pool(name="h1pool", bufs=nb * HC))
    rppool = ctx.enter_context(tc.tile_pool(name="rppool", bufs=nb))
    bvec = ctx.enter_context(tc.tile_pool(name="bvec", bufs=nb))
    sqj = ctx.enter_context(tc.tile_pool(name="sqj", bufs=2))
    tmppool = ctx.enter_context(tc.tile_pool(name="tmppool", bufs=4))
    h2pool = ctx.enter_context(tc.tile_pool(name="h2pool", bufs=2))
    opool = ctx.enter_context(tc.tile_pool(name="opool", bufs=2))
    psA = ctx.enter_context(tc.tile_pool(name="psA", bufs=4, space="PSUM"))
    psB = ctx.enter_context(tc.tile_pool(name="psB", bufs=4, space="PSUM"))
    dramp = ctx.enter_context(tc.tile_pool(name="dramp", bufs=nb, space="DRAM"))

    # ---- one-time constants ------------------------------------------------
    ident = singles.tile([128, 128], BF16)
    make_identity(nc, ident)
    identf = singles.tile([128, 128], F32)
    make_identity(nc, identf)
    ident2 = singles.tile([128, 2, 128], FP8)
    nc.gpsimd.memset(ident2, 0.0)
    make_identity(nc, ident2[:, 0, :], nomemset=True)
    make_identity(nc, ident2[:, 1, :], nomemset=True)
    ones = singles.tile([128, 128], BF16)
    nc.gpsimd.memset(ones, 1.0)
    # eyerows[k, it, :] == 1 iff k == it: K=8 row-selector for broadcasts
    eyerows = singles.tile([NT, NT, 128], BF16)
    nc.gpsimd.memset(eyerows, 0.0)
    nc.gpsimd.affine_select(out=eyerows, in_=eyerows,
                            compare_op=mybir.AluOpType.not_equal, fill=1.0,
                            base=0, pattern=[[-1, NT], [0, 128]],
                            channel_multiplier=1)
    ceps = singles.tile([128, 1], F32)
    nc.gpsimd.memset(ceps, COS_EPS * COS_EPS)
    cneps = singles.tile([128, 1], F32)
    nc.gpsimd.memset(cneps, NORM_EPS * NORM_EPS)

    b1col = singles.tile([128, HC], F32)
    b2col = singles.tile([128, 1], F32)
    w18 = singles.tile([128, KDR, 2, D_H], FP8)
    w1r8 = singles.tile([128, KDR, 2, D_H], FP8)
    w2t = [singles.tile([128, D_OUT], BF16, tag=f"w2t{k}", name=f"w2t{k}")
           for k in range(HC)]

    def load_weights():
        # emitted after the per-graph X loads so those win the DMA queues
        nc.scalar.dma_start(out=b1col, in_=bass.AP(
            tensor=b1.tensor, offset=b1.offset, ap=[[1, 128], [128, HC]]))
        nc.scalar.dma_start(out=b2col, in_=bass.AP(
            tensor=b2.tensor, offset=b2.offset, ap=[[1, 128], [1, 1]]))
        nc.sync.dma_start(out=w18, in_=bass.AP(
            tensor=W18.tensor, offset=W18.offset,
            ap=[[2 * D_H, 128], [256 * D_H, KDR], [D_H, 2], [1, D_H]]))
        nc.sync.dma_start(out=w1r8, in_=bass.AP(
            tensor=W1R8.tensor, offset=W1R8.offset,
            ap=[[2 * D_H, 128], [256 * D_H, KDR], [D_H, 2], [1, D_H]]))
        for k in range(HC):
            nc.scalar.dma_start(out=w2t[k], in_=W2T[k * 128:(k + 1) * 128, :])

    t2 = KNN_THRESHOLD * KNN_THRESHOLD

    # ---- per-phase emitters ------------------------------------------------
    def phase_load(g: _G):
        g.x8 = xpool.tile([128, KDR, 2, N], FP8, tag="x8")
        nc.sync.dma_start(out=g.x8, in_=_xdr_load_ap(g.X8b))
        g.r8 = xpool.tile([128, KDR, 2, N], FP8, tag="r8", bufs=nb)
        nc.scalar.dma_start(out=g.r8, in_=_xdr_load_ap(g.R8b))

    def phase_norm(g: _G):
        # row norms from gram diagonal blocks; produce t*n_j (bounced to a
        # partition-replicated row) and rc_i = 1/n_i (per-partition scalars)
        g.ssqv = bvec.tile([128, NT], F32, tag="ssqv")
        for it in range(NT):
            psd = psB.tile([128, 128], F32, tag="psB", name="psd")
            blk = slice(it * 128, (it + 1) * 128)
            for k in range(KDR):
                nc.tensor.matmul(psd, lhsT=g.x8[:, k, :, blk],
                                 rhs=g.x8[:, k, :, blk],
                                 start=(k == 0), stop=(k == KDR - 1),
                                 perf_mode=DR)
            dj = sqj.tile([128, 128], BF16, tag="dj")
            nc.vector.scalar_tensor_tensor(
                out=dj, in0=psd, scalar=1.0, in1=identf,
                op0=ALU.bypass, op1=ALU.mult,
                accum_out=g.ssqv[:, it:it + 1])
        g.nct = bvec.tile([128, NT], BF16, tag="nct")
        nc.scalar.activation(out=g.nct, in_=g.ssqv, func=AF.Sqrt, scale=t2)
        nclp = bvec.tile([128, NT], F32, tag="nclp")
        nc.scalar.activation(out=nclp, in_=g.ssqv, func=AF.Sqrt, bias=ceps)
        g.rc = bvec.tile([128, NT], F32, tag="rc")
        nc.vector.reciprocal(out=g.rc, in_=nclp)
        # transpose [128, NT] -> [NT, 128] so the DRAM spill is 8 contiguous
        # rows (a partition-major spill would be a 4-byte-descriptor grind)
        psn = psB.tile([128, 128], BF16, tag="psB", name="psn")
        nc.tensor.transpose(psn[:NT, :], g.nct, ident)
        nctT = sqj.tile([NT, 128], BF16, tag="nctT")
        nc.scalar.copy(out=nctT, in_=psn[:NT, :])
        # replicate row it of nctT across all partitions with K=1 matmuls:
        # no DRAM round trip on the startup critical path
        g.nrep = rppool.tile([128, N], BF16, tag="nrep")
        for nh in range(2):
            psr = psB.tile([128, 512], F32, tag="psB", name="psr")
            for itl in range(4):
                it = nh * 4 + itl
                nc.tensor.matmul(psr[:, itl * 128:(itl + 1) * 128],
                                 lhsT=eyerows[:, it, :], rhs=nctT,
                                 start=True, stop=True)
            nc.scalar.copy(out=g.nrep[:, nh * 512:(nh + 1) * 512], in_=psr)

    def phase_gram(g: _G):
        # G row tiles -> threshold -> A in fp8 DR-packed tiles, deg fused
        g.degv = bvec.tile([128, 2 * NT], F32, tag="degv")
        for it in range(NT):
            jd, i = it // 2, it % 2
            blk = slice(it * 128, (it + 1) * 128)
            for jh in range(2):
                ps = psA.tile([128, 512], F32, tag="psA")
                for k in range(KDR):
                    nc.tensor.matmul(
                        ps, lhsT=g.x8[:, k, :, blk],
                        rhs=g.x8[:, k, :, jh * 512:(jh + 1) * 512],
                        start=(k == 0), stop=(k == KDR - 1), perf_mode=DR)
                nc.vector.scalar_tensor_tensor(
                    out=g.at[jd][:, i, jh * 512:(jh + 1) * 512], in0=ps,
                    scalar=g.rc[:, it:it + 1],
                    in1=g.nrep[:, jh * 512:(jh + 1) * 512],
                    op0=ALU.mult, op1=ALU.is_gt,
                    accum_out=g.degv[:, jh * NT + it:jh * NT + it + 1])
        for jd in range(NJD):
            # self loops: add I to both diag blocks of the DR pair in one op
            sl = g.at[jd][:, 0, 2 * jd * 128:2 * jd * 128 + 128]
            dview = bass.AP(tensor=sl.tensor, offset=sl.offset,
                            ap=[list(sl.ap[0]), [N + 128, 2], [1, 128]])
            nc.gpsimd.tensor_add(out=dview, in0=dview, in1=ident2)

        dsum = bvec.tile([128, NT], F32, tag="dsum")
        nc.vector.tensor_tensor(out=dsum, in0=g.degv[:, 0:NT],
                                in1=g.degv[:, NT:2 * NT], op=ALU.add)
        sqd = bvec.tile([128, NT], F32, tag="sqd")
        nc.scalar.activation(out=sqd, in_=dsum, func=AF.Sqrt, bias=1.0)
        g.dv = bvec.tile([128, NT], F32, tag="dv")
        nc.vector.reciprocal(out=g.dv, in_=sqd)
        g.dvw = bvec.tile([128, NT], F32, tag="dvw")
        nc.vector.tensor_scalar_mul(g.dvw, g.dv, Y_SCALE / W_SCALE)
        g.dvy = bvec.tile([128, NT], F32, tag="dvy")
        nc.vector.tensor_scalar_mul(g.dvy, g.dv, Y_SCALE)
        g.dvb = bvec.tile([128, NT], BF16, tag="dvb")
        nc.vector.tensor_scalar_mul(g.dvb, g.dv, 1.0 / Y_SCALE)

    def phase_drep(g: _G):
        # bounce d/Y_SCALE to a partition-replicated row (transposed spill,
        # emitted after xw1 so the tensor engine never waits on the chain)
        psv = psB.tile([128, 128], BF16, tag="psB", name="psv")
        nc.tensor.transpose(psv[:NT, :], g.dvb, ident)
        dvT = sqj.tile([NT, 128], BF16, tag="dvT")
        nc.scalar.copy(out=dvT, in_=psv[:NT, :])
        dscr = dramp.tile([1, N], BF16, tag="dscr")
        dflat = dscr[0]
        nc.gpsimd.dma_start(
            out=bass.AP(tensor=dflat.tensor, offset=dflat.offset,
                        ap=[[128, NT], [1, 128]]),
            in_=dvT)
        g.drep = rppool.tile([128, N], BF16, tag="drep")
        nc.gpsimd.dma_start(out=g.drep, in_=_bcast_p(dflat))

    def phase_xw1(g: _G):
        # G1 = X @ W1.T via compensated fp8: X8@W18 + X8@W1r8 + R8@W18;
        # evict d_j-scaled as fp8 pair (y8, ry) for the DR propagation.
        for it in range(NT):
            jd, i = it // 2, it % 2
            blk = slice(it * 128, (it + 1) * 128)
            ps = psB.tile([128, D_H], F32, tag="psB")
            n9 = 3 * KDR
            step = 0
            for k in range(KDR):
                for lt, rt in ((g.x8, w18), (g.x8, w1r8), (g.r8, w18)):
                    mm = nc.tensor.matmul(ps, lhsT=lt[:, k, :, blk],
                                          rhs=rt[:, k], start=(step == 0),
                                          stop=(step == n9 - 1), perf_mode=DR)
                    if lt is g.x8 and rt is w1r8:
                        # same stationary as the preceding matmul of this
                        # accumulation group: skip the redundant weight load
                        mm.ins.ldweights = False
                    step += 1
            y8sl = g.y8[:, jd, i, :]
            nc.scalar.activation(out=y8sl, in_=ps, func=AF.Copy,
                                 scale=g.dvw[:, it:it + 1])
            nc.vector.scalar_tensor_tensor(
                out=g.ry[:, jd, i, :], in0=ps, scalar=g.dvw[:, it:it + 1],
                in1=y8sl, op0=ALU.mult, op1=ALU.subtract)

    def phase_prop1(g: _G):
        # M1^T = (A diag(d) G1)^T via DR pairs; H1^T = relu(d_i * M1^T + b1)
        pss = {}
        for hc in range(HC):
            g.h1t.append(h1pool.tile([128, N], BF16, tag="h1", name="h1"))
            for ih in range(2):
                pss[hc, ih] = psA.tile([128, 512], F32, tag="psA", name="psd2")
        nsrc = 2 * NJD
        step = 0
        for jd in range(NJD):
            for src in (g.y8, g.ry):
                st = step == 0
                sp = step == nsrc - 1
                step += 1
                for hc in range(HC):
                    lhsT = src[:, jd, :, hc * 128:(hc + 1) * 128]
                    for ih in range(2):
                        nc.tensor.matmul(
                            pss[hc, ih], lhsT=lhsT,
                            rhs=g.at[jd][:, :, ih * 512:(ih + 1) * 512],
                            start=st, stop=sp, perf_mode=DR)
        for hc in range(HC):
            for ih in range(2):
                tmp = tmppool.tile([128, 512], F32, tag="tmp")
                nc.vector.tensor_tensor(out=tmp, in0=pss[hc, ih],
                                        in1=g.drep[:, ih * 512:(ih + 1) * 512],
                                        op=ALU.mult)
                nc.scalar.activation(out=g.h1t[hc][:, ih * 512:(ih + 1) * 512],
                                     in_=tmp, func=AF.Relu,
                                     bias=b1col[:, hc:hc + 1])

    def phase_e(g: _G):
        # Ys2 = d_j * (H1 @ W2.T), evicted as fp8 pair (y2, r2)
        for it in range(NT):
            jd, i = it // 2, it % 2
            ps = psB.tile([128, D_OUT], F32, tag="psB", name="psE")
            for hc in range(HC):
                nc.tensor.matmul(ps, lhsT=g.h1t[hc][:, it * 128:(it + 1) * 128],
                                 rhs=w2t[hc], start=(hc == 0),
                                 stop=(hc == HC - 1))
            y2sl = g.y2[:, jd, i, :]
            nc.scalar.activation(out=y2sl, in_=ps, func=AF.Copy,
                                 scale=g.dvy[:, it:it + 1])
            nc.vector.scalar_tensor_tensor(
                out=g.r2[:, jd, i, :], in0=ps, scalar=g.dvy[:, it:it + 1],
                in1=y2sl, op0=ALU.mult, op1=ALU.subtract)

    def phase_f1(g: _G, ih: int):
        # H2^T half = (A Ys2)^T in wide DR matmuls; per-node 1/norm derived
        # on-chip via transposes of the partition-replicated ssq row.
        if ih == 0:
            g.h2tb = h2pool.tile([128, N], BF16, tag="h2tb", bufs=nb)
            g.rl = bvec.tile([128, NT], F32, tag="rl")
        nsrc = 2 * NJD
        ps2 = psA.tile([128, 512], F32, tag="psA", name="ps2")
        step = 0
        for jd in range(NJD):
            for src in (g.y2, g.r2):
                nc.tensor.matmul(
                    ps2, lhsT=src[:, jd],
                    rhs=g.at[jd][:, :, ih * 512:(ih + 1) * 512],
                    start=(step == 0), stop=(step == nsrc - 1),
                    perf_mode=DR)
                step += 1
        half = slice(ih * 512, (ih + 1) * 512)
        tmp = tmppool.tile([128, 512], F32, tag="tmp")
        nc.vector.tensor_tensor(out=tmp, in0=ps2, in1=g.drep[:, half],
                                op=ALU.mult)
        nc.vector.tensor_scalar(out=g.h2tb[:, half], in0=tmp, scalar1=b2col,
                                scalar2=None, op0=ALU.add)
        sq = sqj.tile([128, 512], BF16, tag="sq")
        nc.vector.tensor_tensor(out=sq, in0=g.h2tb[:, half],
                                in1=g.h2tb[:, half], op=ALU.mult)
        pssq = psB.tile([128, 512], F32, tag="psB", name="pssq")
        nc.tensor.matmul(pssq, lhsT=ones, rhs=sq, start=True, stop=True)
        sqs = sqj.tile([128, 512], BF16, tag="sqs")
        nc.scalar.copy(out=sqs, in_=pssq)
        # every row of sqs is the same ssq vector, so transposing a [128,128]
        # block turns column n into the per-partition scalar layout
        psq = psB.tile([128, 512], BF16, tag="psB", name="psq")
        for itl in range(4):
            nc.tensor.transpose(psq[:, itl * 128:(itl + 1) * 128],
                                sqs[:, itl * 128:(itl + 1) * 128], ident)
        rsl = psq[:, 0:1]
        nc.scalar.copy(
            out=g.rl[:, ih * 4:(ih + 1) * 4],
            in_=bass.AP(tensor=rsl.tensor, offset=rsl.offset,
                        ap=[list(rsl.ap[0]), [128, 4]]))

    def phase_f2(g: _G, ih: int):
        # per-node 1/max(norm, eps), then PE transposes back to [n, dout]
        # with the scale applied during the DVE eviction
        srl = bvec.tile([128, NT // 2], F32, tag="srl")
        nc.scalar.activation(out=srl, in_=g.rl[:, ih * 4:(ih + 1) * 4],
                             func=AF.Sqrt, bias=cneps)
        rli = bvec.tile([128, NT // 2], F32, tag="rli")
        nc.vector.reciprocal(out=rli, in_=srl)
        if ih == 0:
            g.obuf = opool.tile([128, NT * D_OUT], F32, tag="obuf")
        for itl in range(4):
            it = ih * 4 + itl
            pst = psB.tile([128, 128], BF16, tag="psB", name="pst")
            nc.tensor.transpose(pst, g.h2tb[:, it * 128:(it + 1) * 128], ident)
            nc.vector.tensor_scalar(out=g.obuf[:, it * 128:(it + 1) * 128],
                                    in0=pst, scalar1=rli[:, itl:itl + 1],
                                    scalar2=None, op0=ALU.mult)
        nc.sync.dma_start(
            out=bass.AP(tensor=g.Yb.tensor,
                        offset=g.Yb.offset + ih * 512 * D_OUT,
                        ap=[[D_OUT, 128], [128 * D_OUT, NT // 2], [1, D_OUT]]),
            in_=g.obuf[:, ih * 512:(ih + 1) * 512])

    # ---- wave-pipelined driver ---------------------------------------------
    gs = []
    for bi in range(nb):
        g = _G()
        g.X8b, g.R8b, g.Yb = X8[bi], R8[bi], Y[bi]
        g.h1t = []
        g.at = []
        gs.append(g)

    for g in gs:
        phase_load(g)
    load_weights()
    for g in gs:
        # A tiles allocated up front so the threshold can write DR slices
        for jd in range(NJD):
            g.at.append(apool.tile([128, 2, N], FP8, tag="at", bufs=nb * NJD,
                                   name="at"))
        g.y8 = ypool.tile([128, NJD, 2, D_H], FP8, tag="y8")
        g.ry = ypool.tile([128, NJD, 2, D_H], FP8, tag="ry", bufs=nb)
        g.y2 = ypool.tile([128, NJD, 2, D_OUT], FP8, tag="y2", bufs=nb)
        g.r2 = ypool.tile([128, NJD, 2, D_OUT], FP8, tag="r2", bufs=nb)
    for g in gs:
        phase_norm(g)
    # xw1(g-1) is emitted between gram(g-1) and gram(g) so its DVE residual
    # evictions drain while the tensor engine streams the next graph's gram
    phase_gram(gs[0])
    for gi in range(1, nb):
        phase_xw1(gs[gi - 1])
        phase_drep(gs[gi - 1])
        phase_gram(gs[gi])
    phase_xw1(gs[nb - 1])
    phase_drep(gs[nb - 1])
    for g in gs:
        phase_prop1(g)
    # phase_e emissions are interleaved into the f pipeline as tensor-engine
    # filler while each half's normalize chain drains on vector/scalar/gpsimd
    phase_e(gs[0])
    phase_e(gs[1])
    halves = [(g, ih) for g in gs for ih in range(2)]
    nh = len(halves)
    for i in range(nh):
        phase_f1(*halves[i])
        if i % 2 == 1 and i // 2 + 2 < nb:
            phase_e(gs[i // 2 + 2])
        if i >= 2:
            phase_f2(*halves[i - 2])
    phase_f2(*halves[nh - 2])
    phase_f2(*halves[nh - 1])
